# revision 8
# baseline (speedup 1.0000x reference)
"""FDLoss kernel for Trainium2 (Bass/Tile), data-parallel over 8 NeuronCores.

Math (a = target.flatten(), b = source.flatten()):
    fdback = where(a<0 & b<0, b-a, a-b)
    loss   = mean((fdback - a)^2)
Per element:  d = b + relu(-2a)*(b<0);  loss = mean(d^2)

Inputs quantize to fp8 e4m3 on host (~1e-3 rel err, 4x less HBM traffic;
engines upconvert fp8->fp32 on read). A single custom DVE op for everything
is 1 elem/cycle @0.96GHz ~= 54us; this version splits the element stream:

  custom path (n1=27136 cols): fused DVE op d^2 = sq(b + relu(-2a)*(b<0)),
      free-dim accum -> partials column. ~1.07 ns/col on DVE.
  offload path (n2=23040 cols): exact identity
      d^2 = (b + w2)^2 = b^2 + w2^2 + 2*b*w2,
      w2  = min(relu(-2a), relu(-2^20 b))   [= 2*relu(-a)*[b<0]]
      ACT (+GpSimd probe): u2 = relu(-2a), vK = relu(-2^20 b) (bf16 out;
           EXACT — fp8 mantissa fits bf16, scales are pow2).
      DVE: w2 = min(u2, vK) — bf16 tensor_tensor in 2x mode, 0.53 ns/col.
      PE : per 128-col block, 3 gram matmuls accumulated into PSUM tiles:
           psB += b.T@b, psW += w2.T@w2, psX += w2.T@b.
           Host: partials.sum + tr(psB) + tr(psW) + 2*tr(psX).

Scheduling notes (evolved across perfetto traces: 96 -> 90 -> 66 -> this):
  - Dedicated SBUF tile per input transfer; buffer-reuse WAR waits on a
    dma_start stall the whole FIFO HWDGE ring behind it.
  - ALL input DMAs on the sync ring only, in exact joint consumption
    order (offload tiles split into a-half/b-half transfers so the ACT
    can start on the a-half early). One ring = strict FIFO at the full
    aggregate SDMA rate; two busy rings round-robin at half rate each.
  - Tiny DVE "fence" copies (read w2, write the custom-path scratch)
    force the greedy scheduler to keep the mins interleaved with the
    custom ops; otherwise they all sink to the end of the DVE program.
  - Custom op sizes ramp up at the start (small ops while DMA ramps),
    and the interleave matches ACT's ~7.2us/chunk cadence; the tail ends
    with min6 + a small custom so the last W/X matmuls and PSUM copies
    overlap the final DVE work.
  - b-gram matmuls only need input DMAs: emitted first, keeps PE warm.
  - GpSimd runs the u2-relu for two mid chunks (tensor_scalar mult+max):
    probes GPS throughput and SBUF-port contention with the DVE.
"""

from operator import add as _operator_add

import numpy as np
import ml_dtypes

import concourse.bacc as bacc
import concourse.mybir as mybir
import concourse.dve_ops as dve_ops
from concourse.dve_ops import DveOp
from concourse.dve_spec import Spec, Src0, Src1, C0, Zero, relu, sq, lower, _has_src1
from concourse.dve_uop import DveOpSpec
from concourse.tile import TileContext
from concourse.bass_utils import run_bass_kernel_spmd

N_CORES = 8
FULL_SHAPE = (64, 256, 56, 56)
TOTAL = 64 * 256 * 56 * 56          # 51,380,224
PER_CORE = TOTAL // N_CORES         # 6,422,528 = 128 * 50,176
P = 128
FD_TOTAL = PER_CORE // P            # 50,176 pair-columns per partition

# ---------------------------------------------------------------------------
# Custom-path tiles and the DVE op slices within them (ramp, then big ops,
# then a taper so the DVE stream ends on small ops).
_CUST_TILES = [1024, 2048, 4096, 6656, 6656, 4480]
_CUST_OPS = [
    [256, 256, 512],
    [1024, 1024],
    [2048, 2048],
    [3328, 3328],
    [3328, 3328],
    [2304, 1280, 896],
]
DVE_TOTAL = sum(_CUST_TILES)        # 24,960
assert [sum(o) for o in _CUST_OPS] == _CUST_TILES
_N_CUST_OPS = sum(len(o) for o in _CUST_OPS)   # 14
N_COLS = _N_CUST_OPS

# Offload chunks (ACT/min granularity == input tile granularity)
_OFF_SIZES = [2048, 4096, 4096, 4096, 4096, 4096, 2048, 640]
OFF_TOTAL = sum(_OFF_SIZES)         # 25,216
N_OFF_CHUNKS = len(_OFF_SIZES)
assert DVE_TOTAL + OFF_TOTAL == FD_TOTAL
assert all(s % 128 == 0 for s in _OFF_SIZES)

# Offload chunks whose u2-relu runs on GpSimd instead of ACT. Probed with
# {2, 4}: a 4096-col fp8->bf16 tensor_scalar took 58us PER OP on GpSimd
# (~14 cyc/elem software fp8 path) and the shared SBUF port degraded the
# DVE from 45us busy to 159us. GpSimd must stay COMPLETELY idle.
_GPS_U = set()


def _gen_dve_order():
    """Greedy interleave of custom ops (fixed tile/op order) with the mins,
    paced by the modeled ACT chunk cadence so each min lands just after its
    relu pair completes (measured: ACT act = (m+352)/1.2 ns, custom op
    ~1.16 ns/col, min = (58+m/2)/0.96 + fence 0.17us, DVE start ~10.4us,
    ACT first relu ~9.6us)."""
    cust = [("c", i, j) for i, ops in enumerate(_CUST_OPS) for j in range(len(ops))]
    act_t = 9.6e3
    act_end = []
    for m in _OFF_SIZES:
        act_t += 2 * (m + 352) / 1.2
        act_end.append(act_t)
    order = []
    dve_t = 10.4e3
    ci = 0
    for c, m in enumerate(_OFF_SIZES):
        while ci < len(cust) and dve_t + 400 < act_end[c]:
            n = _CUST_OPS[cust[ci][1]][cust[ci][2]]
            order.append(cust[ci])
            dve_t += n * 1.16 + 160
            ci += 1
        order.append(("min", c))
        dve_t = max(dve_t, act_end[c]) + (58 + m / 2) / 0.96 + 170
    order.extend(cust[ci:])
    return order


_DVE_ORDER = _gen_dve_order()
assert sorted(x[1:] for x in _DVE_ORDER if x[0] == "c") == sorted(
    (i, j) for i in range(len(_CUST_TILES)) for j in range(len(_CUST_OPS[i]))
)
assert sorted(x[1] for x in _DVE_ORDER if x[0] == "min") == list(range(N_OFF_CHUNKS))

# DMA issue order = joint consumption order. ("c", i) = custom tile i whole;
# ("offa", c) / ("offb", c) = a-half / b-half of offload chunk c.
_ISSUE = [
    ("c", 0), ("offa", 0), ("offb", 0),
    ("c", 1), ("c", 2),
    ("offa", 1), ("offb", 1),
    ("offa", 2), ("c", 3), ("offb", 2),
    ("offa", 3), ("offb", 3),
    ("c", 4),
    ("offa", 4), ("offb", 4),
    ("offa", 5), ("offb", 5),
    ("c", 5),
    ("offa", 6), ("offb", 6),
    ("offa", 7), ("offb", 7),
]
assert sorted(i for p, i in _ISSUE if p == "c") == list(range(len(_CUST_TILES)))
assert sorted(i for p, i in _ISSUE if p == "offa") == list(range(N_OFF_CHUNKS))
assert sorted(i for p, i in _ISSUE if p == "offb") == list(range(N_OFF_CHUNKS))

_F32 = mybir.dt.float32
_BF16 = mybir.dt.bfloat16
_F8 = mybir.dt.float8e4
_F8_NP = ml_dtypes.float8_e4m3

_VSCALE = float(2 ** 20)
_OP_NAME = "FDLOSS_SQ_REDUCE"
_OFF_MAX = max(_OFF_SIZES)
_WT_MAX = 3328


def _fdloss_ref(in0, in1, c0, c1, c2):
    a = np.asarray(in0).astype(np.float32)
    bb = np.asarray(in1).astype(np.float32)
    b = np.square(bb + np.maximum(a * c0, 0.0) * (bb < 0.0)).astype(np.float32)
    return b, b.reshape(b.shape[0], -1).sum(axis=-1, keepdims=True)


def _register_op() -> DveOp:
    for op in dve_ops.OPS:
        if op.name == _OP_NAME:
            return op
    spec = Spec(
        body=sq(Src1 + relu(Src0 * C0) * (Src1 < Zero)),
        accum=_operator_add,
        accum_init=Zero,
        reference=_fdloss_ref,
    )
    row = dve_ops._CUSTOM_DVE_ROW_BASE + len(dve_ops.OPS)
    shas = {}
    for ver in ("v3", "v4"):
        compiled = DveOpSpec(
            name=_OP_NAME,
            opcode=row,
            uops=lower(spec, ver=ver),
            rd1_en=_has_src1(spec),
        )
        shas[ver] = compiled.sha(ver)
    op = DveOp(_OP_NAME, spec, subdim=False, uops_sha=shas)
    dve_ops.OPS.append(op)
    dve_ops._SUB_OPCODE_FOR_NAME[_OP_NAME] = row
    dve_ops.CUSTOM_DVE_SPECS[_OP_NAME] = spec
    return op


_cached_nc = None


def _build_bass():
    fd_op = _register_op()
    nc = bacc.Bacc(trn_type="TRN2")

    ab_d = nc.dram_tensor("ab_in", (2 * PER_CORE,), _F8, kind="ExternalInput")
    out_d = nc.dram_tensor("partials", (P, N_COLS), _F32, kind="ExternalOutput")
    gram_d = nc.dram_tensor("gram", (P, 3 * P), _F32, kind="ExternalOutput")

    relu_fn = mybir.ActivationFunctionType.Relu
    min_op = mybir.AluOpType.min
    mult_op = mybir.AluOpType.mult
    max_op = mybir.AluOpType.max

    with TileContext(nc) as tc:
        import contextlib

        stack = contextlib.ExitStack()
        with stack:
            in_pool = stack.enter_context(tc.tile_pool(name="inp", bufs=1))
            u_pool = stack.enter_context(tc.tile_pool(name="u", bufs=3))
            v_pool = stack.enter_context(tc.tile_pool(name="v", bufs=3))
            w_pool = stack.enter_context(tc.tile_pool(name="w", bufs=3))
            misc_pool = stack.enter_context(tc.tile_pool(name="misc", bufs=1))
            ps_pool = stack.enter_context(tc.tile_pool(name="ps", bufs=1, space="PSUM"))

            acc = misc_pool.tile([P, N_COLS], _F32)
            wt = misc_pool.tile([P, _WT_MAX], _F32)   # write-only DVE scratch
            warm = misc_pool.tile([P, 8], _BF16)      # ACT table warmup target
            gram_sb = misc_pool.tile([P, 3 * P], _F32)
            psB = ps_pool.tile([P, P], _F32)
            psW = ps_pool.tile([P, P], _F32)
            psX = ps_pool.tile([P, P], _F32)

            cust_tiles = [
                in_pool.tile([P, 2 * n], _F8, name=f"cust{i}")
                for i, n in enumerate(_CUST_TILES)
            ]
            off_tiles = [
                in_pool.tile([P, 2 * n], _F8, name=f"off{i}")
                for i, n in enumerate(_OFF_SIZES)
            ]

            # ---- ACT warmup FIRST on the scalar queue.
            nc.scalar.activation(out=warm[:, :8], in_=warm[:, :8], func=relu_fn)

            # ---- all input DMAs on the SYNC ring, consumption order.
            elem_off = 0
            for path, idx in _ISSUE:
                if path == "c":
                    n = _CUST_TILES[idx]
                    t = cust_tiles[idx]
                    src = ab_d[elem_off : elem_off + P * 2 * n].rearrange(
                        "(p m) -> p m", p=P
                    )
                    elem_off += P * 2 * n
                    nc.sync.dma_start(out=t[:, :], in_=src)
                else:
                    m = _OFF_SIZES[idx]
                    t = off_tiles[idx]
                    src = ab_d[elem_off : elem_off + P * m].rearrange(
                        "(p m) -> p m", p=P
                    )
                    elem_off += P * m
                    if path == "offa":
                        nc.sync.dma_start(out=t[:, :m], in_=src)
                    else:
                        nc.sync.dma_start(out=t[:, m : 2 * m], in_=src)

            # ---- unary relus: ACT (+GpSimd for probed chunks' u2).
            u_tiles = {}
            v_tiles = {}
            for c in range(N_OFF_CHUNKS):
                m = _OFF_SIZES[c]
                abt = off_tiles[c]
                ut = u_pool.tile([P, _OFF_MAX], _BF16, tag="u")
                vt = v_pool.tile([P, _OFF_MAX], _BF16, tag="v")
                if c in _GPS_U:
                    nc.gpsimd.tensor_scalar(
                        out=ut[:, :m], in0=abt[:, :m],
                        scalar1=-2.0, scalar2=0.0, op0=mult_op, op1=max_op,
                    )
                else:
                    nc.scalar.activation(
                        out=ut[:, :m], in_=abt[:, :m], func=relu_fn, scale=-2.0
                    )
                nc.scalar.activation(
                    out=vt[:, :m], in_=abt[:, m : 2 * m], func=relu_fn, scale=-_VSCALE
                )
                u_tiles[c] = ut
                v_tiles[c] = vt

            # ---- PE: b-gram matmuls (depend only on input DMAs) first.
            first_b = True
            for c in range(N_OFF_CHUNKS):
                m = _OFF_SIZES[c]
                abt = off_tiles[c]
                for j in range(m // P):
                    b_ap = abt[:, m + j * P : m + (j + 1) * P]
                    nc.tensor.matmul(
                        out=psB[:, :], lhsT=b_ap, rhs=b_ap,
                        start=first_b,
                        stop=(c == N_OFF_CHUNKS - 1 and j == m // P - 1),
                    )
                    first_b = False

            # ---- DVE stream (+ dependent PE grams) in consumption order.
            cust_off = [[0] * len(ops) for ops in _CUST_OPS]
            for i, ops in enumerate(_CUST_OPS):
                o = 0
                for j, n in enumerate(ops):
                    cust_off[i][j] = o
                    o += 2 * n
            col = 0
            first_w = True
            for item in _DVE_ORDER:
                if item[0] == "c":
                    _, i, j = item
                    n = _CUST_OPS[i][j]
                    o = cust_off[i][j]
                    t = cust_tiles[i]
                    nc.vector._custom_dve(
                        fd_op,
                        out=wt[:, :n],
                        in0=t[:, o : o + n],
                        in1=t[:, o + n : o + 2 * n],
                        s0=-2.0,
                        accum_out=acc[:, col : col + 1],
                    )
                    col += 1
                else:
                    c = item[1]
                    m = _OFF_SIZES[c]
                    abt = off_tiles[c]
                    ut, vt = u_tiles[c], v_tiles[c]
                    w2 = w_pool.tile([P, _OFF_MAX], _BF16, tag="w")
                    nc.vector.tensor_tensor(
                        out=w2[:, :m], in0=ut[:, :m], in1=vt[:, :m], op=min_op
                    )
                    # fence: WAW on wt orders every later DVE op after this min
                    nc.vector.tensor_copy(out=wt[:, 0:8], in_=w2[:, 0:8])
                    last_c = c == N_OFF_CHUNKS - 1
                    for j in range(m // P):
                        w_ap = w2[:, j * P : (j + 1) * P]
                        b_ap = abt[:, m + j * P : m + (j + 1) * P]
                        last_j = last_c and j == m // P - 1
                        nc.tensor.matmul(
                            out=psW[:, :], lhsT=w_ap, rhs=w_ap,
                            start=first_w, stop=last_j,
                        )
                        nc.tensor.matmul(
                            out=psX[:, :], lhsT=w_ap, rhs=b_ap,
                            start=first_w, stop=last_j,
                        )
                        first_w = False
            assert col == N_COLS

            # ---- tail: PSUM -> SBUF -> DRAM, plus the custom partials.
            nc.scalar.copy(out=gram_sb[:, 0:P], in_=psB[:, :])
            nc.scalar.copy(out=gram_sb[:, P : 2 * P], in_=psW[:, :])
            nc.scalar.copy(out=gram_sb[:, 2 * P : 3 * P], in_=psX[:, :])
            k = N_COLS - 1
            nc.scalar.dma_start(out=out_d[:, :k], in_=acc[:, :k])
            nc.sync.dma_start(out=gram_d[:, :], in_=gram_sb[:, :])
            nc.sync.dma_start(out=out_d[:, k:], in_=acc[:, k:], single_packet=True)

    nc.compile()
    return nc


def _get_nc():
    global _cached_nc
    if _cached_nc is None:
        _cached_nc = _build_bass()
    return _cached_nc


def _pack_inputs(source, target):
    """Quantize to fp8 and pack per-core flat arrays in _ISSUE order.
    Custom tiles hold consecutive [P, 2, n_op] blocks per DVE op; offload
    a-half transfers are [P, m] of a, b-halves [P, m] of b."""
    a = np.asarray(target, dtype=np.float32).reshape(N_CORES, P, FD_TOTAL)
    b = np.asarray(source, dtype=np.float32).reshape(N_CORES, P, FD_TOTAL)
    a = a.astype(_F8_NP)
    b = b.astype(_F8_NP)

    cust_base = [0] * len(_CUST_TILES)
    o = 0
    for i, n in enumerate(_CUST_TILES):
        cust_base[i] = o
        o += n
    off_base = [0] * N_OFF_CHUNKS
    o = DVE_TOTAL
    for i, n in enumerate(_OFF_SIZES):
        off_base[i] = o
        o += n

    packed = np.empty((N_CORES, 2 * PER_CORE), dtype=_F8_NP)
    elem_off = 0
    for path, idx in _ISSUE:
        if path == "c":
            o = cust_base[idx]
            for ns in _CUST_OPS[idx]:
                blk = np.stack([a[:, :, o : o + ns], b[:, :, o : o + ns]], axis=2)
                packed[:, elem_off : elem_off + P * 2 * ns] = blk.reshape(N_CORES, -1)
                elem_off += P * 2 * ns
                o += ns
        else:
            m = _OFF_SIZES[idx]
            o = off_base[idx]
            srcarr = a if path == "offa" else b
            packed[:, elem_off : elem_off + P * m] = srcarr[
                :, :, o : o + m
            ].reshape(N_CORES, -1)
            elem_off += P * m
    assert elem_off == 2 * PER_CORE
    return packed


def kernel_impl(source, target, trace=False, **run_kwargs):
    packed = _pack_inputs(source, target)
    in_maps = [{"ab_in": packed[i]} for i in range(N_CORES)]

    nc = _get_nc()
    res = run_bass_kernel_spmd(
        nc, in_maps, core_ids=list(range(N_CORES)), trace=trace, **run_kwargs
    )
    total = np.float64(0.0)
    for r in res.results:
        total += r["partials"].astype(np.float64).sum()
        g = r["gram"].astype(np.float64)
        total += np.trace(g[:, 0:P])
        total += np.trace(g[:, P : 2 * P])
        total += 2.0 * np.trace(g[:, 2 * P : 3 * P])
    loss = np.float32(total / TOTAL)
    return np.array(loss, dtype=np.float32), res


def kernel(**inputs) -> np.ndarray:
    out, _ = kernel_impl(inputs["source"], inputs["target"])
    return out


# revision 11
# speedup vs baseline: 1.0395x; 1.0395x over previous
"""FDLoss kernel for Trainium2 (Bass/Tile), data-parallel over 8 NeuronCores.

Math (a = target.flatten(), b = source.flatten()):
    fdback = where(a<0 & b<0, b-a, a-b)
    loss   = mean((fdback - a)^2)
Per element:  d = b + relu(-2a)*(b<0);  loss = mean(d^2)

Inputs quantize to fp8 e4m3 on host (~1e-3 rel err, 4x less HBM traffic;
engines upconvert fp8->fp32 on read). A single custom DVE op for everything
is 1 elem/cycle @0.96GHz ~= 54us; this version splits the element stream:

  custom path (n1=27136 cols): fused DVE op d^2 = sq(b + relu(-2a)*(b<0)),
      free-dim accum -> partials column. ~1.07 ns/col on DVE.
  offload path (n2=23040 cols): exact identity
      d^2 = (b + w2)^2 = b^2 + w2^2 + 2*b*w2,
      w2  = min(relu(-2a), relu(-2^20 b))   [= 2*relu(-a)*[b<0]]
      ACT (+GpSimd probe): u2 = relu(-2a), vK = relu(-2^20 b) (bf16 out;
           EXACT — fp8 mantissa fits bf16, scales are pow2).
      DVE: w2 = min(u2, vK) — bf16 tensor_tensor in 2x mode, 0.53 ns/col.
      PE : per 128-col block, 3 gram matmuls accumulated into PSUM tiles:
           psB += b.T@b, psW += w2.T@w2, psX += w2.T@b.
           Host: partials.sum + tr(psB) + tr(psW) + 2*tr(psX).

Scheduling notes (evolved across perfetto traces: 96 -> 90 -> 66 -> this):
  - Dedicated SBUF tile per input transfer; buffer-reuse WAR waits on a
    dma_start stall the whole FIFO HWDGE ring behind it.
  - ALL input DMAs on the sync ring only, in exact joint consumption
    order (offload tiles split into a-half/b-half transfers so the ACT
    can start on the a-half early). One ring = strict FIFO at the full
    aggregate SDMA rate; two busy rings round-robin at half rate each.
  - Tiny DVE "fence" copies (read w2, write the custom-path scratch)
    force the greedy scheduler to keep the mins interleaved with the
    custom ops; otherwise they all sink to the end of the DVE program.
  - Custom op sizes ramp up at the start (small ops while DMA ramps),
    and the interleave matches ACT's ~7.2us/chunk cadence; the tail ends
    with min6 + a small custom so the last W/X matmuls and PSUM copies
    overlap the final DVE work.
  - b-gram matmuls only need input DMAs: emitted first, keeps PE warm.
  - GpSimd runs the u2-relu for two mid chunks (tensor_scalar mult+max):
    probes GPS throughput and SBUF-port contention with the DVE.
"""

from operator import add as _operator_add

import numpy as np
import ml_dtypes

import concourse.bacc as bacc
import concourse.mybir as mybir
import concourse.dve_ops as dve_ops
from concourse.dve_ops import DveOp
from concourse.dve_spec import Spec, Src0, Src1, C0, Zero, relu, sq, lower, _has_src1
from concourse.dve_uop import DveOpSpec
from concourse.tile import TileContext
from concourse.bass_utils import run_bass_kernel_spmd

N_CORES = 8
FULL_SHAPE = (64, 256, 56, 56)
TOTAL = 64 * 256 * 56 * 56          # 51,380,224
PER_CORE = TOTAL // N_CORES         # 6,422,528 = 128 * 50,176
P = 128
FD_TOTAL = PER_CORE // P            # 50,176 pair-columns per partition

# ---------------------------------------------------------------------------
# Custom-path tiles and the DVE op slices within them (ramp, then big ops,
# then a taper so the DVE stream ends on small ops).
_CUST_TILES = [1024, 2048, 4096, 6656, 6656, 6656]
_CUST_OPS = [
    [256, 256, 512],
    [1024, 1024],
    [2048, 2048],
    [3328, 3328],
    [3328, 3328],
    [2560, 2432, 1664],
]
DVE_TOTAL = sum(_CUST_TILES)        # 27,136
assert [sum(o) for o in _CUST_OPS] == _CUST_TILES
_N_CUST_OPS = sum(len(o) for o in _CUST_OPS)   # 15
N_COLS = _N_CUST_OPS

# Offload chunks (ACT/min granularity == input tile granularity). Small
# first chunks (early DMA supply is tight), small last chunks (their W/X
# matmuls + PSUM copies are the kernel tail).
_OFF_SIZES = [2048, 2048, 4096, 4096, 4096, 4096, 1536, 1024]
OFF_TOTAL = sum(_OFF_SIZES)         # 23,040
N_OFF_CHUNKS = len(_OFF_SIZES)
assert DVE_TOTAL + OFF_TOTAL == FD_TOTAL
assert all(s % 128 == 0 for s in _OFF_SIZES)

# Offload chunks whose u2-relu runs on GpSimd instead of ACT. Probed with
# {2, 4}: a 4096-col fp8->bf16 tensor_scalar took 58us PER OP on GpSimd
# (~14 cyc/elem software fp8 path) and the shared SBUF port degraded the
# DVE from 45us busy to 159us. GpSimd must stay COMPLETELY idle.
_GPS_U = set()


def _gen_dve_order():
    """Greedy interleave of custom ops (fixed tile/op order) with the mins,
    paced by the modeled ACT chunk cadence so each min lands just after its
    relu pair completes (measured: ACT act = (m+352)/1.2 ns, custom op
    ~1.16 ns/col, min = (58+m/2)/0.96 + fence 0.17us, DVE start ~10.4us,
    ACT first relu ~9.6us)."""
    cust = [("c", i, j) for i, ops in enumerate(_CUST_OPS) for j in range(len(ops))]
    act_t = 10.2e3
    act_end = []
    for m in _OFF_SIZES:
        act_t += 2 * (m + 352) / 1.2
        act_end.append(act_t)
    order = []
    dve_t = 10.4e3
    ci = 0
    for c, m in enumerate(_OFF_SIZES):
        while ci < len(cust) and dve_t + 400 < act_end[c]:
            n = _CUST_OPS[cust[ci][1]][cust[ci][2]]
            order.append(cust[ci])
            dve_t += n * 1.16 + 160
            ci += 1
        order.append(("min", c))
        dve_t = max(dve_t, act_end[c]) + (58 + m / 2) / 0.96 + 170
    order.extend(cust[ci:])
    return order


_DVE_ORDER = _gen_dve_order()
assert sorted(x[1:] for x in _DVE_ORDER if x[0] == "c") == sorted(
    (i, j) for i in range(len(_CUST_TILES)) for j in range(len(_CUST_OPS[i]))
)
assert sorted(x[1] for x in _DVE_ORDER if x[0] == "min") == list(range(N_OFF_CHUNKS))

# DMA issue order = joint consumption order. ("c", i) = custom tile i whole;
# ("offa", c) / ("offb", c) = a-half / b-half of offload chunk c.
_ISSUE = [
    ("c", 0), ("offa", 0), ("offb", 0),
    ("c", 1), ("offa", 1), ("offb", 1),
    ("c", 2), ("c", 3),
    ("offa", 2), ("offb", 2),
    ("c", 4),
    ("offa", 3), ("offb", 3),
    ("offa", 4), ("offb", 4),
    ("c", 5),
    ("offa", 5), ("offb", 5),
    ("offa", 6), ("offb", 6),
    ("offa", 7), ("offb", 7),
]
assert sorted(i for p, i in _ISSUE if p == "c") == list(range(len(_CUST_TILES)))
assert sorted(i for p, i in _ISSUE if p == "offa") == list(range(N_OFF_CHUNKS))
assert sorted(i for p, i in _ISSUE if p == "offb") == list(range(N_OFF_CHUNKS))

_F32 = mybir.dt.float32
_BF16 = mybir.dt.bfloat16
_F8 = mybir.dt.float8e4
_F8_NP = ml_dtypes.float8_e4m3

_VSCALE = float(2 ** 20)
_OP_NAME = "FDLOSS_SQ_REDUCE"
_OFF_MAX = max(_OFF_SIZES)
_WT_MAX = 3328


def _fdloss_ref(in0, in1, c0, c1, c2):
    a = np.asarray(in0).astype(np.float32)
    bb = np.asarray(in1).astype(np.float32)
    b = np.square(bb + np.maximum(a * c0, 0.0) * (bb < 0.0)).astype(np.float32)
    return b, b.reshape(b.shape[0], -1).sum(axis=-1, keepdims=True)


def _register_op() -> DveOp:
    for op in dve_ops.OPS:
        if op.name == _OP_NAME:
            return op
    spec = Spec(
        body=sq(Src1 + relu(Src0 * C0) * (Src1 < Zero)),
        accum=_operator_add,
        accum_init=Zero,
        reference=_fdloss_ref,
    )
    row = dve_ops._CUSTOM_DVE_ROW_BASE + len(dve_ops.OPS)
    shas = {}
    for ver in ("v3", "v4"):
        compiled = DveOpSpec(
            name=_OP_NAME,
            opcode=row,
            uops=lower(spec, ver=ver),
            rd1_en=_has_src1(spec),
        )
        shas[ver] = compiled.sha(ver)
    op = DveOp(_OP_NAME, spec, subdim=False, uops_sha=shas)
    dve_ops.OPS.append(op)
    dve_ops._SUB_OPCODE_FOR_NAME[_OP_NAME] = row
    dve_ops.CUSTOM_DVE_SPECS[_OP_NAME] = spec
    return op


_cached_nc = None


def _build_bass():
    fd_op = _register_op()
    nc = bacc.Bacc(trn_type="TRN2")

    ab_d = nc.dram_tensor("ab_in", (2 * PER_CORE,), _F8, kind="ExternalInput")
    out_d = nc.dram_tensor("partials", (P, N_COLS), _F32, kind="ExternalOutput")
    gram_d = nc.dram_tensor("gram", (P, 3 * P), _F32, kind="ExternalOutput")

    relu_fn = mybir.ActivationFunctionType.Relu
    min_op = mybir.AluOpType.min
    mult_op = mybir.AluOpType.mult
    max_op = mybir.AluOpType.max

    with TileContext(nc) as tc:
        import contextlib

        stack = contextlib.ExitStack()
        with stack:
            in_pool = stack.enter_context(tc.tile_pool(name="inp", bufs=1))
            u_pool = stack.enter_context(tc.tile_pool(name="u", bufs=3))
            v_pool = stack.enter_context(tc.tile_pool(name="v", bufs=3))
            w_pool = stack.enter_context(tc.tile_pool(name="w", bufs=3))
            misc_pool = stack.enter_context(tc.tile_pool(name="misc", bufs=1))
            ps_pool = stack.enter_context(tc.tile_pool(name="ps", bufs=1, space="PSUM"))

            acc = misc_pool.tile([P, N_COLS], _F32)
            wt = misc_pool.tile([P, _WT_MAX], _F32)   # write-only DVE scratch
            warm = misc_pool.tile([P, 8], _BF16)      # ACT table warmup target
            gram_sb = misc_pool.tile([P, 3 * P], _F32)
            psB = ps_pool.tile([P, P], _F32)
            psW = ps_pool.tile([P, P], _F32)
            psX = ps_pool.tile([P, P], _F32)

            cust_tiles = [
                in_pool.tile([P, 2 * n], _F8, name=f"cust{i}")
                for i, n in enumerate(_CUST_TILES)
            ]
            off_tiles = [
                in_pool.tile([P, 2 * n], _F8, name=f"off{i}")
                for i, n in enumerate(_OFF_SIZES)
            ]

            # ---- ACT warmup FIRST on the scalar queue.
            nc.scalar.activation(out=warm[:, :8], in_=warm[:, :8], func=relu_fn)

            # ---- all input DMAs on the SYNC ring, consumption order.
            elem_off = 0
            for path, idx in _ISSUE:
                if path == "c":
                    n = _CUST_TILES[idx]
                    t = cust_tiles[idx]
                    src = ab_d[elem_off : elem_off + P * 2 * n].rearrange(
                        "(p m) -> p m", p=P
                    )
                    elem_off += P * 2 * n
                    nc.sync.dma_start(out=t[:, :], in_=src)
                else:
                    m = _OFF_SIZES[idx]
                    t = off_tiles[idx]
                    src = ab_d[elem_off : elem_off + P * m].rearrange(
                        "(p m) -> p m", p=P
                    )
                    elem_off += P * m
                    if path == "offa":
                        nc.sync.dma_start(out=t[:, :m], in_=src)
                    else:
                        nc.sync.dma_start(out=t[:, m : 2 * m], in_=src)

            # ---- unary relus: ACT (+GpSimd for probed chunks' u2).
            u_tiles = {}
            v_tiles = {}
            for c in range(N_OFF_CHUNKS):
                m = _OFF_SIZES[c]
                abt = off_tiles[c]
                ut = u_pool.tile([P, _OFF_MAX], _BF16, tag="u")
                vt = v_pool.tile([P, _OFF_MAX], _BF16, tag="v")
                if c in _GPS_U:
                    nc.gpsimd.tensor_scalar(
                        out=ut[:, :m], in0=abt[:, :m],
                        scalar1=-2.0, scalar2=0.0, op0=mult_op, op1=max_op,
                    )
                else:
                    nc.scalar.activation(
                        out=ut[:, :m], in_=abt[:, :m], func=relu_fn, scale=-2.0
                    )
                nc.scalar.activation(
                    out=vt[:, :m], in_=abt[:, m : 2 * m], func=relu_fn, scale=-_VSCALE
                )
                u_tiles[c] = ut
                v_tiles[c] = vt

            # ---- PE: b-gram matmuls (depend only on input DMAs) first.
            first_b = True
            for c in range(N_OFF_CHUNKS):
                m = _OFF_SIZES[c]
                abt = off_tiles[c]
                for j in range(m // P):
                    b_ap = abt[:, m + j * P : m + (j + 1) * P]
                    nc.tensor.matmul(
                        out=psB[:, :], lhsT=b_ap, rhs=b_ap,
                        start=first_b,
                        stop=(c == N_OFF_CHUNKS - 1 and j == m // P - 1),
                    )
                    first_b = False

            # ---- DVE stream (+ dependent PE grams) in consumption order.
            cust_off = [[0] * len(ops) for ops in _CUST_OPS]
            for i, ops in enumerate(_CUST_OPS):
                o = 0
                for j, n in enumerate(ops):
                    cust_off[i][j] = o
                    o += 2 * n
            col = 0
            first_w = True
            for item in _DVE_ORDER:
                if item[0] == "c":
                    _, i, j = item
                    n = _CUST_OPS[i][j]
                    o = cust_off[i][j]
                    t = cust_tiles[i]
                    nc.vector._custom_dve(
                        fd_op,
                        out=wt[:, :n],
                        in0=t[:, o : o + n],
                        in1=t[:, o + n : o + 2 * n],
                        s0=-2.0,
                        accum_out=acc[:, col : col + 1],
                    )
                    col += 1
                else:
                    c = item[1]
                    m = _OFF_SIZES[c]
                    abt = off_tiles[c]
                    ut, vt = u_tiles[c], v_tiles[c]
                    w2 = w_pool.tile([P, _OFF_MAX], _BF16, tag="w")
                    nc.vector.tensor_tensor(
                        out=w2[:, :m], in0=ut[:, :m], in1=vt[:, :m], op=min_op
                    )
                    # fence: WAW on wt orders every later DVE op after this min
                    nc.vector.tensor_copy(out=wt[:, 0:8], in_=w2[:, 0:8])
                    last_c = c == N_OFF_CHUNKS - 1
                    for j in range(m // P):
                        w_ap = w2[:, j * P : (j + 1) * P]
                        b_ap = abt[:, m + j * P : m + (j + 1) * P]
                        last_j = last_c and j == m // P - 1
                        nc.tensor.matmul(
                            out=psW[:, :], lhsT=w_ap, rhs=w_ap,
                            start=first_w, stop=last_j,
                        )
                        nc.tensor.matmul(
                            out=psX[:, :], lhsT=w_ap, rhs=b_ap,
                            start=first_w, stop=last_j,
                        )
                        first_w = False
            assert col == N_COLS

            # ---- tail: PSUM -> SBUF -> DRAM, plus the custom partials.
            nc.scalar.copy(out=gram_sb[:, 0:P], in_=psB[:, :])
            nc.scalar.copy(out=gram_sb[:, P : 2 * P], in_=psW[:, :])
            nc.scalar.copy(out=gram_sb[:, 2 * P : 3 * P], in_=psX[:, :])
            k = N_COLS - 1
            nc.scalar.dma_start(out=out_d[:, :k], in_=acc[:, :k])
            nc.sync.dma_start(out=gram_d[:, :], in_=gram_sb[:, :])
            nc.sync.dma_start(out=out_d[:, k:], in_=acc[:, k:], single_packet=True)

    nc.compile()
    return nc


def _get_nc():
    global _cached_nc
    if _cached_nc is None:
        _cached_nc = _build_bass()
    return _cached_nc


def _pack_inputs(source, target):
    """Quantize to fp8 and pack per-core flat arrays in _ISSUE order.
    Custom tiles hold consecutive [P, 2, n_op] blocks per DVE op; offload
    a-half transfers are [P, m] of a, b-halves [P, m] of b."""
    a = np.asarray(target, dtype=np.float32).reshape(N_CORES, P, FD_TOTAL)
    b = np.asarray(source, dtype=np.float32).reshape(N_CORES, P, FD_TOTAL)
    a = a.astype(_F8_NP)
    b = b.astype(_F8_NP)

    cust_base = [0] * len(_CUST_TILES)
    o = 0
    for i, n in enumerate(_CUST_TILES):
        cust_base[i] = o
        o += n
    off_base = [0] * N_OFF_CHUNKS
    o = DVE_TOTAL
    for i, n in enumerate(_OFF_SIZES):
        off_base[i] = o
        o += n

    packed = np.empty((N_CORES, 2 * PER_CORE), dtype=_F8_NP)
    elem_off = 0
    for path, idx in _ISSUE:
        if path == "c":
            o = cust_base[idx]
            for ns in _CUST_OPS[idx]:
                blk = np.stack([a[:, :, o : o + ns], b[:, :, o : o + ns]], axis=2)
                packed[:, elem_off : elem_off + P * 2 * ns] = blk.reshape(N_CORES, -1)
                elem_off += P * 2 * ns
                o += ns
        else:
            m = _OFF_SIZES[idx]
            o = off_base[idx]
            srcarr = a if path == "offa" else b
            packed[:, elem_off : elem_off + P * m] = srcarr[
                :, :, o : o + m
            ].reshape(N_CORES, -1)
            elem_off += P * m
    assert elem_off == 2 * PER_CORE
    return packed


def kernel_impl(source, target, trace=False, **run_kwargs):
    packed = _pack_inputs(source, target)
    in_maps = [{"ab_in": packed[i]} for i in range(N_CORES)]

    nc = _get_nc()
    res = run_bass_kernel_spmd(
        nc, in_maps, core_ids=list(range(N_CORES)), trace=trace, **run_kwargs
    )
    total = np.float64(0.0)
    for r in res.results:
        total += r["partials"].astype(np.float64).sum()
        g = r["gram"].astype(np.float64)
        total += np.trace(g[:, 0:P])
        total += np.trace(g[:, P : 2 * P])
        total += 2.0 * np.trace(g[:, 2 * P : 3 * P])
    loss = np.float32(total / TOTAL)
    return np.array(loss, dtype=np.float32), res


def kernel(**inputs) -> np.ndarray:
    out, _ = kernel_impl(inputs["source"], inputs["target"])
    return out
